# revision 32
# baseline (speedup 1.0000x reference)
"""Causal self-attention Trainium2 kernel (8-core head-parallel tensor parallel).

v2 strategy (bf16 dataflow, ACT/PE balanced):
  - 16 heads split across 8 cores (2 heads each).
  - Host prep: x^T bf16 (shared), per-core W_qkv slice^T bf16, per-core
    W_proj column-slice^T bf16, per-core qkv bias slice f32.
  - Device per core, feature-major dataflow, emission interleaved as
    chunk(ch) -> attention(b,qc) -> chunk(ch+1) -> proj(b,qc) -> ...:
      qkv^T = W^T.T @ x^T (+bias via DVE tensor_scalar_add)  [384, BT] bf16
      V transposed per 128-token tile into [k, (V_h0|ones|V_h1)] layout
      per (b, q-chunk):
        per k-tile kt (both heads), diag-trimmed to the causal region:
          S^T[k,q] = K^T.T @ Q^T     (PE bf16, psum [128,2,512] pair)
          expS     = exp(0.125*S^T)  (one ACT op over both heads' banks)
          diag boundary band zeroed post-exp (GpSimd affine_select)
          psy[65,2,512] += [V|1]^T.T @ expS  (PE, row with ones = Z)
        Z -> sbuf (ACT), broadcast 1/.. via one PE matmul (E2), wide DVE
        reciprocal [128,512], 2 DVE muls -> y^T bf16 [128, 512]
      out^T partial = Wp^T.T @ y^T  (PE) -> DVE copy bf16 -> DRAM
  - Host: sum 8 bf16 partials in f32, transpose, +b_proj.
"""

import sys

if "/opt/trn_rl_repo" not in sys.path:
    sys.path.insert(0, "/opt/trn_rl_repo")

import numpy as np

# ---- problem constants (hardcoded for the grading harness) ----
B, T, C, H = 2, 2048, 1024, 16
HD = C // H            # 64
N_CORES = 8
HPC = H // N_CORES     # heads per core = 2


def _cfg_full():
    return dict(B=B, T=T, C=C, HPC=HPC)


def build_nc(cfg):
    """Build the single-core SPMD Bass program."""
    import concourse.bacc as bacc
    import concourse.mybir as mybir
    import concourse.tile as tile
    from concourse.masks import make_identity

    Bc, Tc, Cc, hpc = cfg["B"], cfg["T"], cfg["C"], cfg["HPC"]
    f32 = mybir.dt.float32
    bf16 = mybir.dt.bfloat16
    BT = Bc * Tc
    MQ = hpc * HD                 # rows per m-group (q|k|v) = 128
    assert MQ == 128
    KT_C = Cc // 128              # contraction tiles for qkv/x = 8
    TOKC = 512
    NCH = BT // TOKC              # token chunks over both batches = 8
    QC = Tc // TOKC               # q-chunks per batch = 4
    KTT = Tc // 128               # k-tiles per batch = 16
    MO = Cc // 128                # proj output tiles = 8
    CH_PER_B = Tc // TOKC         # chunks per batch = 4

    nc = bacc.Bacc()
    xT = nc.declare_dram_parameter("xT", [Cc, BT], bf16, isOutput=False)
    wqkvT = nc.declare_dram_parameter("wqkvT", [Cc, 3 * MQ], bf16, isOutput=False)
    bqkv = nc.declare_dram_parameter("bqkv", [3 * MQ, 1], f32, isOutput=False)
    wpT = nc.declare_dram_parameter("wpT", [MQ, Cc], bf16, isOutput=False)
    outT = nc.declare_dram_parameter("outT", [Cc, BT], bf16, isOutput=True)

    xT_r = xT.rearrange("(kt p) t -> p kt t", p=128)
    wq_r = wqkvT.rearrange("(kt p) m -> p kt m", p=128)
    bq_r = bqkv.rearrange("(g p) o -> p (g o)", p=128)

    AF = mybir.ActivationFunctionType

    with tile.TileContext(nc) as tc:
        with (
            tc.tile_pool(name="consts", bufs=1) as consts,
            tc.tile_pool(name="xpool", bufs=3) as xpool,
            tc.tile_pool(name="epool", bufs=5) as epool,
            tc.tile_pool(name="ypool", bufs=2) as ypool,
            tc.tile_pool(name="zpool", bufs=2) as zpool,
            tc.tile_pool(name="rpool", bufs=2) as rpool,
            tc.tile_pool(name="opool", bufs=4) as opool,
            tc.tile_pool(name="ps_s", bufs=2, space="PSUM") as ps_s,
            tc.tile_pool(name="ps_y", bufs=1, space="PSUM") as ps_y,
            tc.tile_pool(name="ps_misc", bufs=2, space="PSUM") as ps_misc,
        ):
            # ---- constants ----
            w_sb = consts.tile([128, KT_C, 3 * MQ], bf16, tag="w")
            b_sb = consts.tile([128, 3], f32, tag="b")
            nc.sync.dma_start(out=b_sb, in_=bq_r)
            # wp is first needed by the proj of block 0, well after startup;
            # load it behind the first x chunk (separate queue position).
            wp_sb = consts.tile([128, Cc], bf16, tag="wp")
            nc.scalar.dma_start(out=wp_sb, in_=wpT[:, :])
            ident = consts.tile([128, 128], f32, tag="ident")
            make_identity(nc, ident)
            ident_bf = consts.tile([128, 128], bf16, tag="ident_bf")
            nc.vector.tensor_copy(ident_bf[:, :], ident[:, :])
            # ones row vector for Z broadcast (rank-1 matmul)
            ones1 = consts.tile([1, HD], bf16, tag="ones1")
            nc.vector.memset(ones1[:, :], 1.0)

            # qkv feature-major buffers [128, BT] bf16
            qT_sb = consts.tile([128, BT], bf16, tag="qT")
            kT_sb = consts.tile([128, BT], bf16, tag="kT")
            vT_sb = consts.tile([128, BT], bf16, tag="vT")
            # transposed V, one tile per (batch, k-tile) so dependency
            # tracking stays precise: [128 tok, (V_h0|ones|V_h1|ones)]
            v_sb = [
                [consts.tile([128, 2 * (HD + 1)], bf16, tag=f"v{b}_{kt}",
                             name=f"v{b}_{kt}") for kt in range(KTT)]
                for b in range(Bc)
            ]
            for b in range(Bc):
                for kt in range(KTT):
                    nc.vector.memset(v_sb[b][kt][:, HD:HD + 1], 1.0)
                    nc.vector.memset(
                        v_sb[b][kt][:, 2 * HD + 1:2 * HD + 2], 1.0)

            # ---- emission helpers ----
            def emit_chunk(ch, finisher=None, leftover=None):
                """QKV projection + V transpose for one 512-token chunk.
                `finisher` (previous block's normalize tail: bcast matmul +
                reciprocal + muls) is emitted after the second m-group so the
                PE reaches it once the z-copy (ACT) has certainly finished.
                `leftover` drains remaining proj steps between m-groups."""
                x_t = xpool.tile([128, KT_C, TOKC], bf16, tag="x")
                for kt in range(KT_C):
                    if ch == 0:
                        # interleave the weight load with the first x chunk so
                        # the first matmul starts after 1/8th of each
                        nc.sync.dma_start(out=w_sb[:, kt, :], in_=wq_r[:, kt, :])
                    nc.sync.dma_start(
                        out=x_t[:, kt, :],
                        in_=xT_r[:, kt, ch * TOKC:(ch + 1) * TOKC])
                b = ch // CH_PER_B
                # v first: its eviction (DVE) finishes during the q/k matmuls
                # so the V transposes below don't stall the PE.
                for m in (2, 0, 1):
                    ps = ps_misc.tile([128, TOKC], f32, tag="mm")
                    for kt in range(KT_C):
                        nc.tensor.matmul(
                            ps[:, :],
                            w_sb[:, kt, m * MQ:(m + 1) * MQ],
                            x_t[:, kt, :],
                            start=(kt == 0), stop=(kt == KT_C - 1),
                        )
                    dst = (qT_sb, kT_sb, vT_sb)[m]
                    nc.vector.tensor_scalar_add(
                        dst[:, ch * TOKC:(ch + 1) * TOKC], ps[:, :],
                        b_sb[:, m:m + 1],
                    )
                    if m == 0 and finisher is not None:
                        finisher()
                        finisher = None
                    if finisher is None and leftover is not None:
                        # leftover proj steps of an older block (its yT muls
                        # were emitted by an earlier finisher, so no hazard)
                        if next(leftover, "done") == "done":
                            leftover = None
                # V transpose for this chunk's 4 k-tiles
                kt0 = (ch % CH_PER_B) * (TOKC // 128)
                for j in range(TOKC // 128):
                    tok = ch * TOKC + j * 128
                    ps_t = ps_misc.tile([128, 128], bf16, tag="mm")
                    nc.tensor.transpose(
                        ps_t[:, :], vT_sb[:, tok:tok + 128], ident_bf[:, :],
                    )
                    nc.vector.tensor_copy(
                        v_sb[b][kt0 + j][:, 0:HD], ps_t[:, 0:HD])
                    nc.vector.tensor_copy(
                        v_sb[b][kt0 + j][:, HD + 1:2 * HD + 1],
                        ps_t[:, HD:2 * HD])
                    # layout per kt: [V_h0(0:64) | ones(64) | V_h1(65:129) | ones(129)]
                # drain any remaining proj steps of the older block
                while leftover is not None:
                    if next(leftover, "done") == "done":
                        leftover = None

            def proj_steps(b, qc, yT, use_act=False):
                """Generator of single proj-tile emissions for one block.
                use_act alternates evictions onto ACT (only safe when ACT has
                no exp backlog, i.e. the final drain)."""
                q_sl = slice(b * Tc + qc * TOKC, b * Tc + (qc + 1) * TOKC)
                for mo in range(MO):
                    pso = ps_misc.tile([128, TOKC], f32, tag="mm")
                    nc.tensor.matmul(
                        pso[:, :], wp_sb[:, mo * 128:(mo + 1) * 128], yT[:, :],
                        start=True, stop=True,
                    )
                    o_t = opool.tile([128, TOKC], bf16, tag="o")
                    if use_act and mo % 2 == 1:
                        nc.scalar.activation(out=o_t[:, :], in_=pso[:, :],
                                             func=AF.Copy)
                    else:
                        nc.vector.tensor_copy(o_t[:, :], pso[:, :])
                    nc.sync.dma_start(
                        out=outT[mo * 128:(mo + 1) * 128, q_sl], in_=o_t[:, :],
                    )
                    yield

            def emit_attention(b, qc, filler):
                """Attention for one (batch, q-chunk); interleaves `filler`
                (previous block's proj steps) into the kt pipeline so the PE
                stays fed while ACT works on exp. Returns yT bf16 tile."""
                n_kt = (qc + 1) * (TOKC // 128)
                q0 = b * Tc + qc * TOKC
                psy = ps_y.tile([128, 2, TOKC], f32, tag="y")
                pend = []   # (kt, lo, e2)

                def emit_av(kt, lo, e2):
                    for hh in range(hpc):
                        nc.tensor.matmul(
                            psy[0:HD + 1, hh, lo:TOKC],
                            v_sb[b][kt][:, (HD + 1) * hh:(HD + 1) * hh + HD + 1],
                            e2[:, hh, lo:TOKC],
                            start=(kt == 0), stop=(kt == n_kt - 1),
                        )

                for kt in range(n_kt):
                    di = kt - qc * (TOKC // 128)
                    lo = 128 * di if di >= 0 else 0
                    ps2 = ps_s.tile([128, 2, TOKC], f32, tag="s")
                    for hh in range(hpc):
                        nc.tensor.matmul(
                            ps2[:, hh, lo:TOKC],
                            kT_sb[HD * hh:HD * (hh + 1),
                                  b * Tc + kt * 128:b * Tc + (kt + 1) * 128],
                            qT_sb[HD * hh:HD * (hh + 1), q0 + lo:q0 + TOKC],
                            start=True, stop=True,
                        )
                    e2 = epool.tile([128, 2, TOKC], bf16, tag="e")
                    nc.scalar.activation(
                        out=e2[:, :, lo:TOKC], in_=ps2[:, :, lo:TOKC],
                        func=AF.Exp, scale=0.125,
                    )
                    if di >= 0:
                        # zero the upper-triangular part of the boundary band
                        nc.gpsimd.affine_select(
                            out=e2[:, :, lo:lo + 128], in_=e2[:, :, lo:lo + 128],
                            compare_op=mybir.AluOpType.is_ge,
                            fill=0.0, base=0,
                            pattern=[[0, 2], [1, 128]],
                            channel_multiplier=-1,
                        )
                    if filler is not None:
                        if next(filler, "done") == "done":
                            filler = None
                    pend.append((kt, lo, e2))
                    if len(pend) > 1:
                        emit_av(*pend.pop(0))
                emit_av(*pend.pop(0))

                # normalize: yT = psy_y / Z  (Z on row 64 for both heads).
                # Only the z-copy (ACT) is emitted here; the PE/DVE tail is
                # returned as a finisher so the PE doesn't stall on the copy.
                z2 = zpool.tile([1, 2, TOKC], bf16, tag="z")
                nc.scalar.activation(out=z2[:, :, :], in_=psy[HD:HD + 1, :, :],
                                     func=AF.Copy)
                yT = ypool.tile([128, TOKC], bf16, tag="yT")

                def finish_normalize():
                    ps_bc = ps_misc.tile([128, TOKC], f32, tag="mm")
                    for hh in range(hpc):
                        nc.tensor.matmul(ps_bc[HD * hh:HD * (hh + 1), :],
                                         ones1[:, :], z2[:, hh, :],
                                         start=True, stop=True)
                    rc = rpool.tile([128, TOKC], f32, tag="rc")
                    nc.vector.reciprocal(rc[:, :], ps_bc[:, :])
                    for hh in range(hpc):
                        nc.vector.tensor_mul(yT[HD * hh:HD * (hh + 1), :],
                                             psy[0:HD, hh, :],
                                             rc[HD * hh:HD * (hh + 1), :])
                return yT, finish_normalize, filler

            # ---- main interleaved schedule ----
            pending = None
            finisher = None
            leftover = None
            for ch in range(NCH):
                emit_chunk(ch, finisher, leftover)
                b, qc = ch // CH_PER_B, ch % CH_PER_B
                yT, finisher, leftover = emit_attention(b, qc, pending)
                if ch < NCH - 1:
                    pending = proj_steps(b, qc, yT)
                else:
                    pending = proj_steps(b, qc, yT, use_act=True)
            finisher()
            if leftover is not None:
                for _ in leftover:
                    pass
            for _ in pending:
                pass

    nc.finalize()
    return nc


def prep_inputs(cfg, x, W_attn, b_attn, W_proj, b_proj):
    """Host-side sharding: returns per-core input dicts."""
    import ml_dtypes
    Bc, Tc, Cc, hpc = cfg["B"], cfg["T"], cfg["C"], cfg["HPC"]
    n_cores = (Cc // HD) // hpc
    BT = Bc * Tc
    MQ = hpc * HD

    x = np.ascontiguousarray(x, dtype=np.float32)
    xT = np.ascontiguousarray(x.reshape(BT, Cc).T).astype(ml_dtypes.bfloat16)

    in_maps = []
    for c in range(n_cores):
        r0 = c * MQ
        rows = []
        for g in range(3):
            rows.append(np.arange(g * Cc + r0, g * Cc + r0 + MQ))
        rows = np.concatenate(rows)
        w_slice = W_attn[rows, :]                       # [384, C]
        wqkvT = np.ascontiguousarray(w_slice.T).astype(ml_dtypes.bfloat16)
        bq = np.ascontiguousarray(b_attn[rows].reshape(MQ * 3, 1))
        wpT = np.ascontiguousarray(W_proj[:, r0:r0 + MQ].T).astype(ml_dtypes.bfloat16)
        in_maps.append({
            "xT": xT,
            "wqkvT": wqkvT,
            "bqkv": bq.astype(np.float32),
            "wpT": wpT,
        })
    return in_maps


def combine(cfg, results, b_proj):
    Bc, Tc, Cc = cfg["B"], cfg["T"], cfg["C"]
    acc = results[0]["outT"].astype(np.float32)
    for r in results[1:]:
        acc = acc + r["outT"].astype(np.float32)
    out = acc.T + b_proj[None, :]
    return np.ascontiguousarray(out.reshape(Bc, Tc, Cc).astype(np.float32))


_NC_CACHE = {}


def kernel(x, W_attn, b_attn, W_proj, b_proj):
    from concourse.bass_utils import run_bass_kernel_spmd

    cfg = _cfg_full()
    key = "full"
    if key not in _NC_CACHE:
        _NC_CACHE[key] = build_nc(cfg)
    nc = _NC_CACHE[key]
    in_maps = prep_inputs(cfg, np.asarray(x), np.asarray(W_attn),
                          np.asarray(b_attn), np.asarray(W_proj),
                          np.asarray(b_proj))
    res = run_bass_kernel_spmd(nc, in_maps, list(range(N_CORES)))
    return combine(cfg, res.results, np.asarray(b_proj, dtype=np.float32))


# revision 34
# speedup vs baseline: 1.0791x; 1.0791x over previous
"""Causal self-attention Trainium2 kernel (8-core head-parallel tensor parallel).

v2 strategy (bf16 dataflow, ACT/PE balanced):
  - 16 heads split across 8 cores (2 heads each).
  - Host prep: x^T bf16 (shared), per-core W_qkv slice^T bf16, per-core
    W_proj column-slice^T bf16, per-core qkv bias slice f32.
  - Device per core, feature-major dataflow, emission interleaved as
    chunk(ch) -> attention(b,qc) -> chunk(ch+1) -> proj(b,qc) -> ...:
      qkv^T = W^T.T @ x^T (+bias via DVE tensor_scalar_add)  [384, BT] bf16
      V transposed per 128-token tile into [k, (V_h0|ones|V_h1)] layout
      per (b, q-chunk):
        per k-tile kt (both heads), diag-trimmed to the causal region:
          S^T[k,q] = K^T.T @ Q^T     (PE bf16, psum [128,2,512] pair)
          expS     = exp(0.125*S^T)  (one ACT op over both heads' banks)
          diag boundary band zeroed post-exp (GpSimd affine_select)
          psy[65,2,512] += [V|1]^T.T @ expS  (PE, row with ones = Z)
        Z -> sbuf (ACT), broadcast 1/.. via one PE matmul (E2), wide DVE
        reciprocal [128,512], 2 DVE muls -> y^T bf16 [128, 512]
      out^T partial = Wp^T.T @ y^T  (PE) -> DVE copy bf16 -> DRAM
  - Host: sum 8 bf16 partials in f32, transpose, +b_proj.
"""

import sys

if "/opt/trn_rl_repo" not in sys.path:
    sys.path.insert(0, "/opt/trn_rl_repo")

import numpy as np

# ---- problem constants (hardcoded for the grading harness) ----
B, T, C, H = 2, 2048, 1024, 16
HD = C // H            # 64
N_CORES = 8
HPC = H // N_CORES     # heads per core = 2


def _cfg_full():
    return dict(B=B, T=T, C=C, HPC=HPC)


def build_nc(cfg):
    """Build the single-core SPMD Bass program."""
    import concourse.bacc as bacc
    import concourse.mybir as mybir
    import concourse.tile as tile
    from concourse.masks import make_identity

    Bc, Tc, Cc, hpc = cfg["B"], cfg["T"], cfg["C"], cfg["HPC"]
    f32 = mybir.dt.float32
    bf16 = mybir.dt.bfloat16
    BT = Bc * Tc
    MQ = hpc * HD                 # rows per m-group (q|k|v) = 128
    assert MQ == 128
    KT_C = Cc // 128              # contraction tiles for qkv/x = 8
    TOKC = 512
    NCH = BT // TOKC              # token chunks over both batches = 8
    QC = Tc // TOKC               # q-chunks per batch = 4
    KTT = Tc // 128               # k-tiles per batch = 16
    MO = Cc // 128                # proj output tiles = 8
    CH_PER_B = Tc // TOKC         # chunks per batch = 4

    nc = bacc.Bacc()
    xT = nc.declare_dram_parameter("xT", [Cc, BT], bf16, isOutput=False)
    wqkvT = nc.declare_dram_parameter("wqkvT", [Cc, 3 * MQ], bf16, isOutput=False)
    bqkv = nc.declare_dram_parameter("bqkv", [3 * MQ, 1], f32, isOutput=False)
    wpT = nc.declare_dram_parameter("wpT", [MQ, Cc], bf16, isOutput=False)
    outT = nc.declare_dram_parameter("outT", [Cc, BT], bf16, isOutput=True)

    xT_r = xT.rearrange("(kt p) t -> p kt t", p=128)
    wq_r = wqkvT.rearrange("(kt p) m -> p kt m", p=128)
    bq_r = bqkv.rearrange("(g p) o -> p (g o)", p=128)

    AF = mybir.ActivationFunctionType

    with tile.TileContext(nc) as tc:
        with (
            tc.tile_pool(name="consts", bufs=1) as consts,
            tc.tile_pool(name="xpool", bufs=3) as xpool,
            tc.tile_pool(name="epool", bufs=5) as epool,
            tc.tile_pool(name="ypool", bufs=2) as ypool,
            tc.tile_pool(name="zpool", bufs=2) as zpool,
            tc.tile_pool(name="rpool", bufs=2) as rpool,
            tc.tile_pool(name="opool", bufs=4) as opool,
            tc.tile_pool(name="ps_s", bufs=2, space="PSUM") as ps_s,
            tc.tile_pool(name="ps_y", bufs=1, space="PSUM") as ps_y,
            tc.tile_pool(name="ps_misc", bufs=2, space="PSUM") as ps_misc,
        ):
            # ---- constants ----
            w_sb = consts.tile([128, KT_C, 3 * MQ], bf16, tag="w")
            b_sb = consts.tile([128, 3], f32, tag="b")
            nc.sync.dma_start(out=b_sb, in_=bq_r)
            # wp is first needed by the proj of block 0, well after startup;
            # load it behind the first x chunk (separate queue position).
            wp_sb = consts.tile([128, Cc], bf16, tag="wp")
            nc.scalar.dma_start(out=wp_sb, in_=wpT[:, :])
            ident = consts.tile([128, 128], f32, tag="ident")
            make_identity(nc, ident)
            ident_bf = consts.tile([128, 128], bf16, tag="ident_bf")
            nc.vector.tensor_copy(ident_bf[:, :], ident[:, :])
            # ones row vector for Z broadcast (rank-1 matmul)
            ones1 = consts.tile([1, HD], bf16, tag="ones1")
            nc.vector.memset(ones1[:, :], 1.0)

            # qkv feature-major buffers [128, BT] bf16
            qT_sb = consts.tile([128, BT], bf16, tag="qT")
            kT_sb = consts.tile([128, BT], bf16, tag="kT")
            vT_sb = consts.tile([128, BT], bf16, tag="vT")
            # transposed V, one tile per (batch, k-tile) so dependency
            # tracking stays precise: [128 tok, (V_h0|ones|V_h1|ones)]
            v_sb = [
                [consts.tile([128, 2 * (HD + 1)], bf16, tag=f"v{b}_{kt}",
                             name=f"v{b}_{kt}") for kt in range(KTT)]
                for b in range(Bc)
            ]
            for b in range(Bc):
                for kt in range(KTT):
                    nc.vector.memset(v_sb[b][kt][:, HD:HD + 1], 1.0)
                    nc.vector.memset(
                        v_sb[b][kt][:, 2 * HD + 1:2 * HD + 2], 1.0)

            # ---- emission helpers ----
            def emit_chunk(ch, finisher=None, leftover=None):
                """QKV projection + V transpose for one 512-token chunk.
                `finisher` (previous block's normalize tail: bcast matmul +
                reciprocal + muls) is emitted after the second m-group so the
                PE reaches it once the z-copy (ACT) has certainly finished.
                `leftover` drains remaining proj steps between m-groups."""
                x_t = xpool.tile([128, KT_C, TOKC], bf16, tag="x")
                for kt in range(KT_C):
                    if ch == 0:
                        # interleave the weight load with the first x chunk so
                        # the first matmul starts after 1/8th of each
                        nc.sync.dma_start(out=w_sb[:, kt, :], in_=wq_r[:, kt, :])
                    nc.sync.dma_start(
                        out=x_t[:, kt, :],
                        in_=xT_r[:, kt, ch * TOKC:(ch + 1) * TOKC])
                b = ch // CH_PER_B
                # v first: its eviction (DVE) finishes during the q/k matmuls
                # so the V transposes below don't stall the PE.
                for m in (2, 0, 1):
                    ps = ps_misc.tile([128, TOKC], f32, tag="mm")
                    for kt in range(KT_C):
                        nc.tensor.matmul(
                            ps[:, :],
                            w_sb[:, kt, m * MQ:(m + 1) * MQ],
                            x_t[:, kt, :],
                            start=(kt == 0), stop=(kt == KT_C - 1),
                        )
                    dst = (qT_sb, kT_sb, vT_sb)[m]
                    nc.vector.tensor_scalar_add(
                        dst[:, ch * TOKC:(ch + 1) * TOKC], ps[:, :],
                        b_sb[:, m:m + 1],
                    )
                    if m == 2 and finisher is not None:
                        finisher()
                        finisher = None
                    if finisher is None and leftover is not None:
                        # leftover proj steps of an older block (its yT muls
                        # were emitted by an earlier finisher, so no hazard)
                        if next(leftover, "done") == "done":
                            leftover = None
                # V transpose for this chunk's 4 k-tiles
                kt0 = (ch % CH_PER_B) * (TOKC // 128)
                for j in range(TOKC // 128):
                    tok = ch * TOKC + j * 128
                    ps_t = ps_misc.tile([128, 128], bf16, tag="mm")
                    nc.tensor.transpose(
                        ps_t[:, :], vT_sb[:, tok:tok + 128], ident_bf[:, :],
                    )
                    nc.vector.tensor_copy(
                        v_sb[b][kt0 + j][:, 0:HD], ps_t[:, 0:HD])
                    nc.vector.tensor_copy(
                        v_sb[b][kt0 + j][:, HD + 1:2 * HD + 1],
                        ps_t[:, HD:2 * HD])
                    # layout per kt: [V_h0(0:64) | ones(64) | V_h1(65:129) | ones(129)]
                # drain any remaining proj steps of the older block
                while leftover is not None:
                    if next(leftover, "done") == "done":
                        leftover = None

            def proj_steps(b, qc, yT, use_act=False):
                """Generator of single proj-tile emissions for one block.
                use_act alternates evictions onto ACT (only safe when ACT has
                no exp backlog, i.e. the final drain)."""
                q_sl = slice(b * Tc + qc * TOKC, b * Tc + (qc + 1) * TOKC)
                for mo in range(MO):
                    pso = ps_misc.tile([128, TOKC], f32, tag="mm")
                    nc.tensor.matmul(
                        pso[:, :], wp_sb[:, mo * 128:(mo + 1) * 128], yT[:, :],
                        start=True, stop=True,
                    )
                    o_t = opool.tile([128, TOKC], bf16, tag="o")
                    if use_act and mo % 2 == 1:
                        nc.scalar.activation(out=o_t[:, :], in_=pso[:, :],
                                             func=AF.Copy)
                    else:
                        nc.vector.tensor_copy(o_t[:, :], pso[:, :])
                    nc.sync.dma_start(
                        out=outT[mo * 128:(mo + 1) * 128, q_sl], in_=o_t[:, :],
                    )
                    yield

            def emit_attention(b, qc, filler):
                """Attention for one (batch, q-chunk); interleaves `filler`
                (previous block's proj steps) into the kt pipeline so the PE
                stays fed while ACT works on exp. Returns yT bf16 tile."""
                n_kt = (qc + 1) * (TOKC // 128)
                q0 = b * Tc + qc * TOKC
                # 1 proj tile per kt step; for short blocks the remainder is
                # returned and drained during the next chunk.
                psy = ps_y.tile([128, 2, TOKC], f32, tag="y")
                pend = []   # (kt, lo, e2)

                def emit_av(kt, lo, e2):
                    for hh in range(hpc):
                        nc.tensor.matmul(
                            psy[0:HD + 1, hh, lo:TOKC],
                            v_sb[b][kt][:, (HD + 1) * hh:(HD + 1) * hh + HD + 1],
                            e2[:, hh, lo:TOKC],
                            start=(kt == 0), stop=(kt == n_kt - 1),
                        )

                for kt in range(n_kt):
                    di = kt - qc * (TOKC // 128)
                    lo = 128 * di if di >= 0 else 0
                    ps2 = ps_s.tile([128, 2, TOKC], f32, tag="s")
                    for hh in range(hpc):
                        nc.tensor.matmul(
                            ps2[:, hh, lo:TOKC],
                            kT_sb[HD * hh:HD * (hh + 1),
                                  b * Tc + kt * 128:b * Tc + (kt + 1) * 128],
                            qT_sb[HD * hh:HD * (hh + 1), q0 + lo:q0 + TOKC],
                            start=True, stop=True,
                        )
                    e2 = epool.tile([128, 2, TOKC], bf16, tag="e")
                    nc.scalar.activation(
                        out=e2[:, :, lo:TOKC], in_=ps2[:, :, lo:TOKC],
                        func=AF.Exp, scale=0.125,
                    )
                    if di >= 0:
                        # zero the upper-triangular part of the boundary band
                        nc.gpsimd.affine_select(
                            out=e2[:, :, lo:lo + 128], in_=e2[:, :, lo:lo + 128],
                            compare_op=mybir.AluOpType.is_ge,
                            fill=0.0, base=0,
                            pattern=[[0, 2], [1, 128]],
                            channel_multiplier=-1,
                        )
                    if filler is not None:
                        if next(filler, "done") == "done":
                            filler = None
                    pend.append((kt, lo, e2))
                    if len(pend) > 1:
                        emit_av(*pend.pop(0))
                emit_av(*pend.pop(0))

                # normalize: yT = psy_y / Z  (Z on row 64 for both heads).
                # Only the z-copy (ACT) is emitted here; the PE/DVE tail is
                # returned as a finisher so the PE doesn't stall on the copy.
                z2 = zpool.tile([1, 2, TOKC], bf16, tag="z")
                nc.scalar.activation(out=z2[:, :, :], in_=psy[HD:HD + 1, :, :],
                                     func=AF.Copy)
                yT = ypool.tile([128, TOKC], bf16, tag="yT")

                def finish_normalize():
                    ps_bc = ps_misc.tile([128, TOKC], f32, tag="mm")
                    for hh in range(hpc):
                        nc.tensor.matmul(ps_bc[HD * hh:HD * (hh + 1), :],
                                         ones1[:, :], z2[:, hh, :],
                                         start=True, stop=True)
                    rc = rpool.tile([128, TOKC], f32, tag="rc")
                    nc.vector.reciprocal(rc[:, :], ps_bc[:, :])
                    for hh in range(hpc):
                        nc.vector.tensor_mul(yT[HD * hh:HD * (hh + 1), :],
                                             psy[0:HD, hh, :],
                                             rc[HD * hh:HD * (hh + 1), :])
                return yT, finish_normalize, filler

            # ---- main interleaved schedule ----
            pending = None
            finisher = None
            leftover = None
            for ch in range(NCH):
                emit_chunk(ch, finisher, leftover)
                b, qc = ch // CH_PER_B, ch % CH_PER_B
                yT, finisher, leftover = emit_attention(b, qc, pending)
                if ch < NCH - 1:
                    pending = proj_steps(b, qc, yT)
                else:
                    pending = proj_steps(b, qc, yT, use_act=True)
            finisher()
            if leftover is not None:
                for _ in leftover:
                    pass
            for _ in pending:
                pass

    nc.finalize()
    return nc


def prep_inputs(cfg, x, W_attn, b_attn, W_proj, b_proj):
    """Host-side sharding: returns per-core input dicts."""
    import ml_dtypes
    Bc, Tc, Cc, hpc = cfg["B"], cfg["T"], cfg["C"], cfg["HPC"]
    n_cores = (Cc // HD) // hpc
    BT = Bc * Tc
    MQ = hpc * HD

    x = np.ascontiguousarray(x, dtype=np.float32)
    xT = np.ascontiguousarray(x.reshape(BT, Cc).T).astype(ml_dtypes.bfloat16)

    in_maps = []
    for c in range(n_cores):
        r0 = c * MQ
        rows = []
        for g in range(3):
            rows.append(np.arange(g * Cc + r0, g * Cc + r0 + MQ))
        rows = np.concatenate(rows)
        w_slice = W_attn[rows, :]                       # [384, C]
        wqkvT = np.ascontiguousarray(w_slice.T).astype(ml_dtypes.bfloat16)
        bq = np.ascontiguousarray(b_attn[rows].reshape(MQ * 3, 1))
        wpT = np.ascontiguousarray(W_proj[:, r0:r0 + MQ].T).astype(ml_dtypes.bfloat16)
        in_maps.append({
            "xT": xT,
            "wqkvT": wqkvT,
            "bqkv": bq.astype(np.float32),
            "wpT": wpT,
        })
    return in_maps


def combine(cfg, results, b_proj):
    Bc, Tc, Cc = cfg["B"], cfg["T"], cfg["C"]
    acc = results[0]["outT"].astype(np.float32)
    for r in results[1:]:
        acc = acc + r["outT"].astype(np.float32)
    out = acc.T + b_proj[None, :]
    return np.ascontiguousarray(out.reshape(Bc, Tc, Cc).astype(np.float32))


_NC_CACHE = {}


def kernel(x, W_attn, b_attn, W_proj, b_proj):
    from concourse.bass_utils import run_bass_kernel_spmd

    cfg = _cfg_full()
    key = "full"
    if key not in _NC_CACHE:
        _NC_CACHE[key] = build_nc(cfg)
    nc = _NC_CACHE[key]
    in_maps = prep_inputs(cfg, np.asarray(x), np.asarray(W_attn),
                          np.asarray(b_attn), np.asarray(W_proj),
                          np.asarray(b_proj))
    res = run_bass_kernel_spmd(nc, in_maps, list(range(N_CORES)))
    return combine(cfg, res.results, np.asarray(b_proj, dtype=np.float32))
